# revision 1
# baseline (speedup 1.0000x reference)
"""Block-causal GQA attention layer on 8 Trainium2 NeuronCores.

Sharding: 8 cores = batch(2) x head-group(4). Core c handles batch b=c//4 and
head group g=c%4 (q heads 4g..4g+3, kv head g). W_attn is column-sharded by
head group, W_proj row-sharded; each core computes a partial [T, C] output and
the host sums the 4 partials per batch element.

Per-core device pipeline (all matmuls fp32r = full-rate tf32-like):
  B) software-pipelined over 16 t-chunks with a 1-chunk lag so the PE stream
     never waits on the ACT/DVE norm+rope chain:
       stage A(i): x DMA, PE-transpose x -> xT, QKV matmuls, RMS stats
       stage B(i-1): rs-scaled copyback (ACT), RoPE (DVE, negative-stride
       half-swap views, norm weights folded into host cos/sin tables),
       PE-transpose q,k -> qT,kT.
  C+D) per 512-wide T-block: scores sT = kT.T @ qT (block-causal lower tiles
     only), exp on ACT (scale=1/sqrt(d)), staircase mask on diagonal tiles,
     PV + all-ones denominator matmuls, approx-reciprocal normalize; then
     immediately the output projection for that T-block and its DMA out.
"""

import numpy as np

import concourse.bacc as bacc
import concourse.bass as bass
import concourse.tile as tile
import concourse.mybir as mybir
from concourse.bass_utils import run_bass_kernel_spmd
from concourse.masks import make_identity

P = 128
T = 2048
C = 2048
N_HEAD = 16
N_KV = 4
HD = 128          # head dim
HG = N_HEAD // N_KV  # heads per group = 4
BLOCK = 16
EPS = 1e-5
ROPE_BASE = 500000.0
QCOLS = HG * HD   # 512 q cols per core
JCOLS = QCOLS + 2 * HD  # 768 qkv cols per core
NT = T // P       # 16 t-chunks
NC16 = C // P     # 16 c-chunks
SCALE = 1.0 / float(np.sqrt(np.float32(HD)))

F32 = mybir.dt.float32
F32R = mybir.dt.float32r
AF = mybir.ActivationFunctionType
ALU = mybir.AluOpType


def build_nc():
    nc = bacc.Bacc("TRN2", target_bir_lowering=False)

    x = nc.dram_tensor("x", [T, C], F32, kind="ExternalInput")
    wa = nc.dram_tensor("wa", [C, JCOLS], F32, kind="ExternalInput")
    wp = nc.dram_tensor("wp", [QCOLS, C], F32, kind="ExternalInput")
    csq = nc.dram_tensor("csq", [T, HD], F32, kind="ExternalInput")
    snq = nc.dram_tensor("snq", [T, HD], F32, kind="ExternalInput")
    csk = nc.dram_tensor("csk", [T, HD], F32, kind="ExternalInput")
    snk = nc.dram_tensor("snk", [T, HD], F32, kind="ExternalInput")
    dm1 = nc.dram_tensor("dm1", [P, P], F32, kind="ExternalInput")
    dm2 = nc.dram_tensor("dm2", [P, 256], F32, kind="ExternalInput")
    out = nc.dram_tensor("out", [T, C], F32, kind="ExternalOutput")

    with tile.TileContext(nc) as tc:
        with tc.tile_pool(name="persist", bufs=1) as persist:
            ident_f = persist.tile([P, P], F32)
            make_identity(nc, ident_f)
            ident = persist.tile([P, P], F32R)
            nc.vector.tensor_copy(ident, ident_f)
            ones_f = persist.tile([P, P], F32)
            nc.vector.memset(ones_f, 1.0)
            ones = persist.tile([P, P], F32R)
            nc.vector.tensor_copy(ones, ones_f)
            dm1_sb = persist.tile([P, P], F32)
            nc.sync.dma_start(dm1_sb, dm1[:])
            dm2_sb = persist.tile([P, 256], F32)
            nc.sync.dma_start(dm2_sb, dm2[:])
            eps_sb = persist.tile([P, 1], F32)
            nc.vector.memset(eps_sb, EPS)

            qT = persist.tile([P, HG, T], F32R)     # [d, h, t]
            kT = persist.tile([P, T], F32R)         # [d, t]
            v_sb = persist.tile([P, NT, HD], F32R)  # [s_in_chunk, s_chunk, d']

            # ---------------- Phase B (pipelined, lag 1) -----------------
            with (
                tc.tile_pool(name="wts", bufs=1) as wts,
                tc.tile_pool(name="bstream", bufs=3) as bstream,
                tc.tile_pool(name="bwork", bufs=4) as bwork,
                tc.tile_pool(name="psB_tp", bufs=2, space="PSUM") as psB_tp,
                tc.tile_pool(name="psB_qa", bufs=2, space="PSUM") as psB_qa,
                tc.tile_pool(name="psB_qb", bufs=2, space="PSUM") as psB_qb,
            ):
                half = HD // 2
                st = {}       # chunk index -> stage-A state dict
                x_tiles = {}

                def dma_x(i):
                    xt = bstream.tile([P, C], F32R, tag="x")
                    nc.sync.dma_start(xt, x[i * P : (i + 1) * P, :].bitcast(F32R))
                    x_tiles[i] = xt

                dma_x(0)
                wa_r = wa[:].rearrange("(co ci) j -> ci co j", ci=P).bitcast(F32R)
                wa_tiles = []

                def load_wa():
                    for ci in range(NC16):
                        wa_ci = wts.tile([P, JCOLS], F32R, tag=f"wa{ci}", name=f"wa{ci}")
                        nc.sync.dma_start(wa_ci, wa_r[:, ci])
                        wa_tiles.append(wa_ci)

                def stageB1(j):
                    """ACT rs-premult + v copy, DVE rope for chunk j."""
                    s = st[j]
                    qa_ps, qb_ps, rs = s["qa"], s["qb"], s["rs"]
                    qrs = bwork.tile([P, QCOLS + HD], F32, tag="qrs")
                    for hh in range(HG + 1):
                        src = (
                            qa_ps[:, hh * HD : (hh + 1) * HD]
                            if hh < HG
                            else qb_ps[:, 0:HD]
                        )
                        nc.scalar.mul(
                            qrs[:, hh * HD : (hh + 1) * HD], src, rs[:, hh : hh + 1]
                        )
                    nc.scalar.copy(v_sb[:, j, :], qb_ps[:, HD : 2 * HD])

                    qswp = bass.AP(
                        tensor=qrs.tensor,
                        offset=qrs.offset + half,
                        ap=[qrs.ap[0], [HD, HG], [-half, 2], [1, half]],
                    )
                    csq_b = bass.AP(
                        tensor=s["csq"].tensor,
                        offset=s["csq"].offset,
                        ap=[s["csq"].ap[0], [0, HG], [1, HD]],
                    )
                    snq_b = bass.AP(
                        tensor=s["snq"].tensor,
                        offset=s["snq"].offset,
                        ap=[s["snq"].ap[0], [0, HG], [half, 2], [1, half]],
                    )
                    t1q = bwork.tile([P, QCOLS], F32, tag="t1q")
                    nc.gpsimd.tensor_tensor(
                        t1q.rearrange("p (h e) -> p h e", h=HG),
                        qrs[:, 0:QCOLS].rearrange("p (h e) -> p h e", h=HG),
                        csq_b,
                        ALU.mult,
                    )
                    t2q = bwork.tile([P, QCOLS], F32, tag="t2q")
                    nc.vector.tensor_tensor(
                        t2q.rearrange("p (h s e) -> p h s e", h=HG, s=2),
                        qswp,
                        snq_b,
                        ALU.mult,
                    )
                    qhat = bwork.tile([P, QCOLS], F32R, tag="qhat")
                    nc.vector.tensor_tensor(qhat, t1q, t2q, ALU.add)

                    kswp = bass.AP(
                        tensor=qrs.tensor,
                        offset=qrs.offset + QCOLS + half,
                        ap=[qrs.ap[0], [-half, 2], [1, half]],
                    )
                    t1k = bwork.tile([P, HD], F32, tag="t1k")
                    nc.gpsimd.tensor_tensor(
                        t1k, qrs[:, QCOLS : QCOLS + HD], s["csk"], ALU.mult
                    )
                    t2k = bwork.tile([P, HD], F32, tag="t2k")
                    nc.vector.tensor_tensor(
                        t2k.rearrange("p (s e) -> p s e", s=2),
                        kswp,
                        s["snk"].rearrange("p (s e) -> p s e", s=2),
                        ALU.mult,
                    )
                    khat = bwork.tile([P, HD], F32R, tag="khat")
                    nc.vector.tensor_tensor(khat, t1k, t2k, ALU.add)
                    s["qhat"], s["khat"] = qhat, khat

                def stageB2(j):
                    """PE transposes of qhat/khat + copyback into qT/kT."""
                    s = st.pop(j)
                    t0 = j * P
                    tqk_ps = psB_tp.tile([P, 640], F32R, tag="tp")
                    for hh in range(HG):
                        nc.tensor.transpose(
                            tqk_ps[:, hh * HD : (hh + 1) * HD],
                            s["qhat"][:, hh * HD : (hh + 1) * HD],
                            ident,
                        )
                    nc.tensor.transpose(tqk_ps[:, QCOLS : QCOLS + HD], s["khat"], ident)
                    nc.vector.tensor_copy(
                        qT[:, :, t0 : t0 + P],
                        tqk_ps[:, 0:QCOLS].rearrange("p (h t) -> p h t", h=HG),
                    )
                    nc.vector.tensor_copy(kT[:, t0 : t0 + P], tqk_ps[:, QCOLS:640])

                xT_tiles = {}

                def emit_tr(i):
                    # rope tables for chunk i (used in stage B1)
                    s = {}
                    t0 = i * P
                    s["csq"] = bstream.tile([P, HD], F32, tag="csq", name="csq_t")
                    nc.sync.dma_start(s["csq"], csq[t0 : t0 + P, :])
                    s["snq"] = bstream.tile([P, HD], F32, tag="snq", name="snq_t")
                    nc.sync.dma_start(s["snq"], snq[t0 : t0 + P, :])
                    s["csk"] = bstream.tile([P, HD], F32, tag="csk", name="csk_t")
                    nc.sync.dma_start(s["csk"], csk[t0 : t0 + P, :])
                    s["snk"] = bstream.tile([P, HD], F32, tag="snk", name="snk_t")
                    nc.sync.dma_start(s["snk"], snk[t0 : t0 + P, :])
                    st[i] = s

                    x_sb = x_tiles.pop(i)
                    xT_sb = bstream.tile([P, NC16, P], F32R, tag="xT")
                    for cg in range(4):
                        tp_ps = psB_tp.tile([P, 640], F32R, tag="tp")
                        for c4 in range(4):
                            ci = 4 * cg + c4
                            nc.tensor.transpose(
                                tp_ps[:, c4 * P : (c4 + 1) * P],
                                x_sb[:, ci * P : (ci + 1) * P],
                                ident,
                            )
                        nc.vector.tensor_copy(
                            xT_sb[:, 4 * cg : 4 * cg + 4, :],
                            tp_ps[:, 0:512].rearrange("p (a b) -> p a b", a=4),
                        )
                    xT_tiles[i] = xT_sb

                emit_tr(0)
                for i in range(NT + 1):
                    if i < NT:
                        if i + 1 < NT:
                            dma_x(i + 1)
                        if i == 0:
                            load_wa()

                        # premult + rope for chunk i-1 (ACT/DVE overlap the MMs)
                        if i >= 1:
                            stageB1(i - 1)

                        # QKV matmuls
                        s = st[i]
                        xT_sb = xT_tiles.pop(i)
                        qa_ps = psB_qa.tile([P, QCOLS], F32, tag="qa")
                        qb_ps = psB_qb.tile([P, 2 * HD], F32, tag="qb")
                        for ci in range(NC16):
                            nc.tensor.matmul(
                                qa_ps,
                                xT_sb[:, ci],
                                wa_tiles[ci][:, 0:QCOLS],
                                start=(ci == 0),
                                stop=(ci == NC16 - 1),
                            )
                            nc.tensor.matmul(
                                qb_ps,
                                xT_sb[:, ci],
                                wa_tiles[ci][:, QCOLS:JCOLS],
                                start=(ci == 0),
                                stop=(ci == NC16 - 1),
                            )
                        s["qa"], s["qb"] = qa_ps, qb_ps

                        # next chunk's x transposes, then chunk i-1 q/k transposes
                        if i + 1 < NT:
                            emit_tr(i + 1)
                        if i >= 1:
                            stageB2(i - 1)

                        # RMS stats
                        ss = bwork.tile([P, HG + 1], F32, tag="ss")
                        for hh in range(HG + 1):
                            src = (
                                qa_ps[:, hh * HD : (hh + 1) * HD]
                                if hh < HG
                                else qb_ps[:, 0:HD]
                            )
                            sq = bwork.tile([P, HD], F32, tag="sq")
                            nc.scalar.activation(
                                sq, src, AF.Square, accum_out=ss[:, hh : hh + 1]
                            )
                        rt = bwork.tile([P, HG + 1], F32, tag="rt")
                        nc.scalar.activation(
                            rt, ss, AF.Sqrt, bias=eps_sb, scale=1.0 / HD
                        )
                        rs = bwork.tile([P, HG + 1], F32, tag="rs")
                        nc.vector.reciprocal(rs, rt)
                        s["rs"] = rs
                    else:
                        stageB1(i - 1)
                        stageB2(i - 1)

            # ---------------- Phase C+D interleaved ----------------------
            OFFS = [0, 128, 256, 256]
            with tc.tile_pool(name="cpersist", bufs=1) as cpersist:
                yT = cpersist.tile([P, HG, T], F32R)  # [d', h, t]
                wp_sb = cpersist.tile([P, HG, C], F32R)
                nc.sync.dma_start(
                    wp_sb, wp[:].rearrange("(h d) e -> d h e", d=P).bitcast(F32R)
                )

                with (
                    tc.tile_pool(name="cwork", bufs=6) as cwork,
                    tc.tile_pool(name="dout", bufs=8) as dout,
                    tc.tile_pool(name="psC_acc", bufs=2, space="PSUM") as psC_acc,
                    tc.tile_pool(name="psC_sc", bufs=4, space="PSUM") as psC_sc,
                ):
                    def emit_proj_part(Tb, part):
                        for tci in [4 * Tb + part]:
                            t0 = tci * P
                            for e in range(4):
                                o_ps = psC_sc.tile([P, 512], F32, tag="sc")
                                for h in range(HG):
                                    nc.tensor.matmul(
                                        o_ps,
                                        yT[:, h, t0 : t0 + P],
                                        wp_sb[:, h, e * 512 : (e + 1) * 512],
                                        start=(h == 0),
                                        stop=(h == HG - 1),
                                    )
                                o_sb = dout.tile([P, 512], F32, tag="o_sb")
                                nc.vector.tensor_copy(o_sb, o_ps)
                                nc.sync.dma_start(
                                    out[t0 : t0 + P, e * 512 : (e + 1) * 512], o_sb
                                )

                    for Ti in range(4):
                        tt0 = Ti * 512
                        for h in range(HG):
                            if h == 1 and Ti >= 1:
                                for part in range(4):
                                    emit_proj_part(Ti - 1, part)
                            yt_ps = psC_acc.tile([P, 512], F32, tag="yt")
                            den_ps = psC_acc.tile([P, 512], F32, tag="den")
                            nS = 4 * Ti + 4
                            for S in range(nS):
                                r = S - 4 * Ti
                                off = OFFS[r] if r >= 0 else 0
                                sc_ps = psC_sc.tile([P, 512], F32, tag="sc")
                                nc.tensor.matmul(
                                    sc_ps[:, off:512],
                                    kT[:, S * P : (S + 1) * P],
                                    qT[:, h, tt0 + off : tt0 + 512],
                                    start=True,
                                    stop=True,
                                )
                                ex = cwork.tile([P, 512], F32R, tag="ex")
                                nc.scalar.activation(
                                    ex[:, off:512], sc_ps[:, off:512], AF.Exp,
                                    scale=SCALE,
                                )
                                if r >= 0:
                                    if r < 3:
                                        nc.vector.tensor_tensor(
                                            ex[:, r * P : (r + 1) * P],
                                            ex[:, r * P : (r + 1) * P],
                                            dm1_sb,
                                            ALU.mult,
                                        )
                                    else:
                                        nc.vector.tensor_tensor(
                                            ex[:, 256:512], ex[:, 256:512], dm2_sb,
                                            ALU.mult,
                                        )
                                nc.tensor.matmul(
                                    yt_ps[:, off:512],
                                    v_sb[:, S, :],
                                    ex[:, off:512],
                                    start=(S == 0),
                                    stop=(S == nS - 1),
                                )
                                nc.tensor.matmul(
                                    den_ps[:, off:512],
                                    ones,
                                    ex[:, off:512],
                                    start=(S == 0),
                                    stop=(S == nS - 1),
                                )
                            denr = cwork.tile([P, 512], F32, tag="denr")
                            scr = cwork.tile([P, 512], F32, tag="scr")
                            nc.vector.reciprocal_approx_accurate(denr, den_ps, scr)
                            nc.vector.tensor_tensor(
                                yT[:, h, tt0 : tt0 + 512], yt_ps, denr, ALU.mult
                            )
                    for part in range(4):
                        emit_proj_part(3, part)



    nc.finalize()
    return nc


def _host_tables(q_norm_w, k_norm_w):
    """RoPE cos/sin tables in [t, d] layout with norm weights folded in."""
    half = HD // 2
    inv_freq = (
        1.0 / (ROPE_BASE ** (np.arange(0, half, dtype=np.float32) / half))
    ).astype(np.float32)
    ang = np.arange(T, dtype=np.float32)[:, None] * inv_freq[None, :]  # [T, half]
    cos = np.cos(ang).astype(np.float32)
    sin = np.sin(ang).astype(np.float32)
    cos2 = np.concatenate([cos, cos], axis=1)           # [T, 128]
    sin2 = np.concatenate([-sin, sin], axis=1)          # [T, 128]
    csq1 = cos2 * q_norm_w[None, :]
    snq1 = sin2 * q_norm_w[None, :]
    csq = np.ascontiguousarray(csq1, dtype=np.float32)  # [T, 128]
    snq = np.ascontiguousarray(snq1, dtype=np.float32)
    csk = (cos2 * k_norm_w[None, :]).astype(np.float32)
    snk = (sin2 * k_norm_w[None, :]).astype(np.float32)
    return csq, snq, csk, snk


def _host_masks():
    idx = np.arange(P)
    stair = (idx[None, :] // BLOCK >= idx[:, None] // BLOCK).astype(np.float32)
    dm1 = stair
    dm2 = np.concatenate([np.zeros((P, P), np.float32), stair], axis=1)
    return np.ascontiguousarray(dm1), np.ascontiguousarray(dm2)


_nc_cache = None


def kernel(x, W_attn, W_proj, q_norm_w, k_norm_w):
    global _nc_cache
    x = np.asarray(x, dtype=np.float32)
    W_attn = np.asarray(W_attn, dtype=np.float32)
    W_proj = np.asarray(W_proj, dtype=np.float32)
    q_norm_w = np.asarray(q_norm_w, dtype=np.float32)
    k_norm_w = np.asarray(k_norm_w, dtype=np.float32)
    B = x.shape[0]

    csq, snq, csk, snk = _host_tables(q_norm_w, k_norm_w)
    dm1, dm2 = _host_masks()

    in_maps = []
    for core in range(8):
        b, g = divmod(core, 4)
        wa_core = np.concatenate(
            [
                W_attn[:, g * QCOLS : (g + 1) * QCOLS],
                W_attn[:, C + g * HD : C + (g + 1) * HD],
                W_attn[:, C + N_KV * HD + g * HD : C + N_KV * HD + (g + 1) * HD],
            ],
            axis=1,
        )
        wp_core = W_proj[g * QCOLS : (g + 1) * QCOLS, :]
        in_maps.append(
            {
                "x": np.ascontiguousarray(x[b]),
                "wa": np.ascontiguousarray(wa_core),
                "wp": np.ascontiguousarray(wp_core),
                "csq": csq, "snq": snq, "csk": csk, "snk": snk,
                "dm1": dm1, "dm2": dm2,
            }
        )

    if _nc_cache is None:
        _nc_cache = build_nc()
    res = run_bass_kernel_spmd(_nc_cache, in_maps, core_ids=list(range(8)))

    out = np.zeros((B, T, C), dtype=np.float32)
    for core in range(8):
        b = core // 4
        out[b] += res.results[core]["out"]
    return out



# revision 20
# speedup vs baseline: 2.6593x; 2.6593x over previous
"""Block-causal GQA attention layer on 8 Trainium2 NeuronCores.

Sharding: 8 cores = batch(2) x head-group(4). Core c handles batch b=c//4 and
head group g=c%4 (q heads 4g..4g+3, kv head g). W_attn is column-sharded by
head group, W_proj row-sharded; each core computes a partial [T, C] output and
the host sums the 4 partials per batch element.

Per-core device pipeline (all matmuls fp32r = full-rate tf32-like):
  B) software-pipelined over 16 t-chunks with a 1-chunk lag so the PE stream
     never waits on the ACT/DVE norm+rope chain:
       stage A(i): x DMA, PE-transpose x -> xT, QKV matmuls, RMS stats
       stage B(i-1): rs-scaled copyback (ACT), RoPE (DVE, negative-stride
       half-swap views, norm weights folded into host cos/sin tables),
       PE-transpose q,k -> qT,kT.
  C+D) per 512-wide T-block: scores sT = kT.T @ qT (block-causal lower tiles
     only), exp on ACT (scale=1/sqrt(d)), staircase mask on diagonal tiles,
     PV + all-ones denominator matmuls, approx-reciprocal normalize; then
     immediately the output projection for that T-block and its DMA out.
"""

import numpy as np

import concourse.bacc as bacc
import concourse.bass as bass
import concourse.tile as tile
import concourse.mybir as mybir
from concourse.bass_utils import run_bass_kernel_spmd
from concourse.masks import make_identity

P = 128
T = 2048
C = 2048
N_HEAD = 16
N_KV = 4
HD = 128          # head dim
HG = N_HEAD // N_KV  # heads per group = 4
BLOCK = 16
EPS = 1e-5
ROPE_BASE = 500000.0
QCOLS = HG * HD   # 512 q cols per core
JCOLS = QCOLS + 2 * HD  # 768 qkv cols per core
NT = T // P       # 16 t-chunks
NC16 = C // P     # 16 c-chunks
SCALE = 1.0 / float(np.sqrt(np.float32(HD)))

F32 = mybir.dt.float32
F32R = mybir.dt.float32r
BF16 = mybir.dt.bfloat16
NPBF16 = mybir.dt.np(mybir.dt.bfloat16)
AF = mybir.ActivationFunctionType
ALU = mybir.AluOpType


def build_nc():
    nc = bacc.Bacc("TRN2", target_bir_lowering=False)

    # xt[i, c, ci, t] = x[i*128 + t, ci*128 + c]: per-t-chunk pre-transposed
    # blocked layout prepared on host; one contiguous [128, 2048] DMA per chunk.
    xt = nc.dram_tensor("xt", [NT, P, NC16, P], BF16, kind="ExternalInput")
    wa = nc.dram_tensor("wa", [C, JCOLS], BF16, kind="ExternalInput")
    wp = nc.dram_tensor("wp", [QCOLS, C], BF16, kind="ExternalInput")
    csq = nc.dram_tensor("csq", [T, HD], F32, kind="ExternalInput")
    snq = nc.dram_tensor("snq", [T, HD], F32, kind="ExternalInput")
    csk = nc.dram_tensor("csk", [T, HD], F32, kind="ExternalInput")
    snk = nc.dram_tensor("snk", [T, HD], F32, kind="ExternalInput")
    dm1 = nc.dram_tensor("dm1", [P, P], BF16, kind="ExternalInput")
    out = nc.dram_tensor("out", [T, C], F32, kind="ExternalOutput")

    with tile.TileContext(nc) as tc:
        with tc.tile_pool(name="persist", bufs=1) as persist:
            ident_f = persist.tile([P, P], F32)
            make_identity(nc, ident_f)
            ident = persist.tile([P, P], F32R)
            nc.vector.tensor_copy(ident, ident_f)
            ones = persist.tile([P, P], BF16)
            nc.vector.memset(ones, 1.0)
            dm1_sb = persist.tile([P, P], BF16)
            nc.sync.dma_start(dm1_sb, dm1[:])
            eps_sb = persist.tile([P, 1], F32)
            nc.vector.memset(eps_sb, EPS)

            qT = persist.tile([P, HG, T], F32R)     # [d, h, t]
            kT = persist.tile([P, T], F32R)         # [d, t]
            v_sb = persist.tile([P, NT, HD], BF16)  # [s_in_chunk, s_chunk, d']
            yT = persist.tile([P, HG, T], BF16)     # [d', h, t]
            wp_sb = persist.tile([P, HG, C], BF16)

            # ---------------- Phase B (pipelined, lag 1) -----------------
            with (
                tc.tile_pool(name="wts", bufs=1) as wts,
                tc.tile_pool(name="bstream", bufs=3) as bstream,
                tc.tile_pool(name="bwork", bufs=4) as bwork,
                tc.tile_pool(name="psB_tp", bufs=2, space="PSUM") as psB_tp,
                tc.tile_pool(name="psB_qa", bufs=2, space="PSUM") as psB_qa,
                tc.tile_pool(name="psB_qb", bufs=2, space="PSUM") as psB_qb,
            ):
                half = HD // 2
                st = {}       # chunk index -> stage-A state dict
                x_tiles = {}

                def dma_x(i):
                    xtl = bstream.tile([P, NC16, P], BF16, tag="x")
                    nc.sync.dma_start(xtl, xt[i])
                    x_tiles[i] = xtl

                dma_x(0)
                wa_r = wa[:].rearrange("(co ci) j -> ci co j", ci=P)
                wa_tiles = []

                def load_wa():
                    for ci in range(NC16):
                        wa_ci = wts.tile([P, JCOLS], BF16, tag=f"wa{ci}", name=f"wa{ci}")
                        nc.sync.dma_start(wa_ci, wa_r[:, ci])
                        wa_tiles.append(wa_ci)

                def stageB1(j):
                    """ACT rs-premult + v copy, DVE rope for chunk j."""
                    s = st[j]
                    qa_ps, qb_ps, rs = s["qa"], s["qb"], s["rs"]
                    qrs = bwork.tile([P, QCOLS + HD], F32, tag="qrs")
                    for hh in range(HG + 1):
                        src = (
                            qa_ps[:, hh * HD : (hh + 1) * HD]
                            if hh < HG
                            else qb_ps[:, 0:HD]
                        )
                        nc.scalar.mul(
                            qrs[:, hh * HD : (hh + 1) * HD], src, rs[:, hh : hh + 1]
                        )
                    nc.scalar.copy(v_sb[:, j, :], qb_ps[:, HD : 2 * HD])

                    qswp = bass.AP(
                        tensor=qrs.tensor,
                        offset=qrs.offset + half,
                        ap=[qrs.ap[0], [HD, HG], [-half, 2], [1, half]],
                    )
                    csq_b = bass.AP(
                        tensor=s["csq"].tensor,
                        offset=s["csq"].offset,
                        ap=[s["csq"].ap[0], [0, HG], [1, HD]],
                    )
                    snq_b = bass.AP(
                        tensor=s["snq"].tensor,
                        offset=s["snq"].offset,
                        ap=[s["snq"].ap[0], [0, HG], [half, 2], [1, half]],
                    )
                    t1q = bwork.tile([P, QCOLS], F32, tag="t1q")
                    nc.gpsimd.tensor_tensor(
                        t1q.rearrange("p (h e) -> p h e", h=HG),
                        qrs[:, 0:QCOLS].rearrange("p (h e) -> p h e", h=HG),
                        csq_b,
                        ALU.mult,
                    )
                    t2q = bwork.tile([P, QCOLS], F32, tag="t2q")
                    nc.vector.tensor_tensor(
                        t2q.rearrange("p (h s e) -> p h s e", h=HG, s=2),
                        qswp,
                        snq_b,
                        ALU.mult,
                    )
                    qhat = bwork.tile([P, QCOLS], F32R, tag="qhat")
                    nc.vector.tensor_tensor(qhat, t1q, t2q, ALU.add)

                    kswp = bass.AP(
                        tensor=qrs.tensor,
                        offset=qrs.offset + QCOLS + half,
                        ap=[qrs.ap[0], [-half, 2], [1, half]],
                    )
                    t1k = bwork.tile([P, HD], F32, tag="t1k")
                    nc.gpsimd.tensor_tensor(
                        t1k, qrs[:, QCOLS : QCOLS + HD], s["csk"], ALU.mult
                    )
                    t2k = bwork.tile([P, HD], F32, tag="t2k")
                    nc.vector.tensor_tensor(
                        t2k.rearrange("p (s e) -> p s e", s=2),
                        kswp,
                        s["snk"].rearrange("p (s e) -> p s e", s=2),
                        ALU.mult,
                    )
                    khat = bwork.tile([P, HD], F32R, tag="khat")
                    nc.vector.tensor_tensor(khat, t1k, t2k, ALU.add)
                    s["qhat"], s["khat"] = qhat, khat

                def stageB2(j):
                    """PE transposes of qhat/khat + copyback into qT/kT."""
                    s = st.pop(j)
                    t0 = j * P
                    tqk_ps = psB_tp.tile([P, 640], F32R, tag="tp")
                    for hh in range(HG):
                        nc.tensor.transpose(
                            tqk_ps[:, hh * HD : (hh + 1) * HD],
                            s["qhat"][:, hh * HD : (hh + 1) * HD],
                            ident,
                        )
                    nc.tensor.transpose(tqk_ps[:, QCOLS : QCOLS + HD], s["khat"], ident)
                    nc.vector.tensor_copy(
                        qT[:, :, t0 : t0 + P],
                        tqk_ps[:, 0:QCOLS].rearrange("p (h t) -> p h t", h=HG),
                    )
                    nc.vector.tensor_copy(kT[:, t0 : t0 + P], tqk_ps[:, QCOLS:640])

                def emit_tr(i):
                    # rope tables for chunk i (used in stage B1)
                    s = {}
                    t0 = i * P
                    s["csq"] = bstream.tile([P, HD], F32, tag="csq", name="csq_t")
                    nc.sync.dma_start(s["csq"], csq[t0 : t0 + P, :])
                    s["snq"] = bstream.tile([P, HD], F32, tag="snq", name="snq_t")
                    nc.sync.dma_start(s["snq"], snq[t0 : t0 + P, :])
                    s["csk"] = bstream.tile([P, HD], F32, tag="csk", name="csk_t")
                    nc.sync.dma_start(s["csk"], csk[t0 : t0 + P, :])
                    s["snk"] = bstream.tile([P, HD], F32, tag="snk", name="snk_t")
                    nc.sync.dma_start(s["snk"], snk[t0 : t0 + P, :])
                    st[i] = s

                emit_tr(0)
                for i in range(NT + 1):
                    if i < NT:
                        if i + 1 < NT:
                            dma_x(i + 1)
                        if i == 0:
                            load_wa()
                        if i == 12:
                            # prefetch wp so phase C isn't gated on it
                            nc.sync.dma_start(
                                wp_sb,
                                wp[:].rearrange("(h d) e -> d h e", d=P),
                            )

                        # premult + rope for chunk i-1 (ACT/DVE overlap the MMs)
                        if i >= 1:
                            stageB1(i - 1)

                        # QKV matmuls
                        s = st[i]
                        xT_sb = x_tiles.pop(i)
                        qa_ps = psB_qa.tile([P, QCOLS], F32, tag="qa")
                        qb_ps = psB_qb.tile([P, 2 * HD], F32, tag="qb")
                        for ci in range(NC16):
                            nc.tensor.matmul(
                                qa_ps,
                                xT_sb[:, ci],
                                wa_tiles[ci][:, 0:QCOLS],
                                start=(ci == 0),
                                stop=(ci == NC16 - 1),
                            )
                            nc.tensor.matmul(
                                qb_ps,
                                xT_sb[:, ci],
                                wa_tiles[ci][:, QCOLS:JCOLS],
                                start=(ci == 0),
                                stop=(ci == NC16 - 1),
                            )
                        s["qa"], s["qb"] = qa_ps, qb_ps

                        # rope tables for next chunk, then chunk i-1 q/k transposes
                        if i + 1 < NT:
                            emit_tr(i + 1)
                        if i >= 1:
                            stageB2(i - 1)

                        # RMS stats
                        ss = bwork.tile([P, HG + 1], F32, tag="ss")
                        for hh in range(HG + 1):
                            src = (
                                qa_ps[:, hh * HD : (hh + 1) * HD]
                                if hh < HG
                                else qb_ps[:, 0:HD]
                            )
                            sq = bwork.tile([P, HD], F32, tag="sq")
                            nc.scalar.activation(
                                sq, src, AF.Square, accum_out=ss[:, hh : hh + 1]
                            )
                        rt = bwork.tile([P, HG + 1], F32, tag="rt")
                        nc.scalar.activation(
                            rt, ss, AF.Sqrt, bias=eps_sb, scale=1.0 / HD
                        )
                        rs = bwork.tile([P, HG + 1], F32, tag="rs")
                        nc.vector.reciprocal(rs, rt)
                        s["rs"] = rs
                    else:
                        stageB1(i - 1)
                        stageB2(i - 1)

            # ---------------- Phase C+D interleaved ----------------------
            # score matmul widths (fp32r needs >=256 cols for full rate);
            # exp/mask/PV/den run on the exact staircase region (bf16, no
            # small-width penalty).
            OFFS = [0, 128, 256, 256]
            if True:
                with (
                    tc.tile_pool(name="cwork", bufs=6) as cwork,
                    tc.tile_pool(name="dout", bufs=8) as dout,
                    tc.tile_pool(name="psC_acc", bufs=2, space="PSUM") as psC_acc,
                    tc.tile_pool(name="psC_sc", bufs=4, space="PSUM") as psC_sc,
                ):
                    def emit_proj_part(Tb, part):
                        for tci in [4 * Tb + part]:
                            t0 = tci * P
                            for e in range(4):
                                o_ps = psC_sc.tile([P, 512], F32, tag="sc")
                                for h in range(HG):
                                    nc.tensor.matmul(
                                        o_ps,
                                        yT[:, h, t0 : t0 + P],
                                        wp_sb[:, h, e * 512 : (e + 1) * 512],
                                        start=(h == 0),
                                        stop=(h == HG - 1),
                                    )
                                o_sb = dout.tile([P, 512], F32, tag="o_sb")
                                nc.vector.tensor_copy(o_sb, o_ps)
                                nc.sync.dma_start(
                                    out[t0 : t0 + P, e * 512 : (e + 1) * 512], o_sb
                                )

                    for Ti in range(4):
                        tt0 = Ti * 512
                        for h in range(HG):
                            if h == 1 and Ti >= 1:
                                for part in range(4):
                                    emit_proj_part(Ti - 1, part)
                            yt_ps = psC_acc.tile([P, 512], F32, tag="yt")
                            den_ps = psC_acc.tile([P, 512], F32, tag="den")
                            nS = 4 * Ti + 4
                            for S in range(nS):
                                r = S - 4 * Ti
                                off = OFFS[r] if r >= 0 else 0
                                eoff = r * P if r >= 0 else 0
                                sc_ps = psC_sc.tile([P, 512], F32, tag="sc")
                                nc.tensor.matmul(
                                    sc_ps[:, off:512],
                                    kT[:, S * P : (S + 1) * P],
                                    qT[:, h, tt0 + off : tt0 + 512],
                                    start=True,
                                    stop=True,
                                )
                                ex = cwork.tile([P, 512], BF16, tag="ex")
                                nc.scalar.activation(
                                    ex[:, eoff:512], sc_ps[:, eoff:512], AF.Exp,
                                    scale=SCALE,
                                )
                                if r >= 0:
                                    nc.vector.tensor_tensor(
                                        ex[:, eoff : eoff + P],
                                        ex[:, eoff : eoff + P],
                                        dm1_sb,
                                        ALU.mult,
                                    )
                                nc.tensor.matmul(
                                    yt_ps[:, eoff:512],
                                    v_sb[:, S, :],
                                    ex[:, eoff:512],
                                    start=(S == 0),
                                    stop=(S == nS - 1),
                                )
                                nc.tensor.matmul(
                                    den_ps[:, eoff:512],
                                    ones,
                                    ex[:, eoff:512],
                                    start=(S == 0),
                                    stop=(S == nS - 1),
                                )
                            denr = cwork.tile([P, 512], F32, tag="denr")
                            scr = cwork.tile([P, 512], F32, tag="scr")
                            nc.vector.reciprocal_approx_accurate(denr, den_ps, scr)
                            nc.vector.tensor_tensor(
                                yT[:, h, tt0 : tt0 + 512], yt_ps, denr, ALU.mult
                            )
                    for part in range(4):
                        emit_proj_part(3, part)



    nc.finalize()
    return nc


def _host_tables(q_norm_w, k_norm_w):
    """RoPE cos/sin tables in [t, d] layout with norm weights folded in."""
    half = HD // 2
    inv_freq = (
        1.0 / (ROPE_BASE ** (np.arange(0, half, dtype=np.float32) / half))
    ).astype(np.float32)
    ang = np.arange(T, dtype=np.float32)[:, None] * inv_freq[None, :]  # [T, half]
    cos = np.cos(ang).astype(np.float32)
    sin = np.sin(ang).astype(np.float32)
    cos2 = np.concatenate([cos, cos], axis=1)           # [T, 128]
    sin2 = np.concatenate([-sin, sin], axis=1)          # [T, 128]
    csq1 = cos2 * q_norm_w[None, :]
    snq1 = sin2 * q_norm_w[None, :]
    csq = np.ascontiguousarray(csq1, dtype=np.float32)  # [T, 128]
    snq = np.ascontiguousarray(snq1, dtype=np.float32)
    csk = (cos2 * k_norm_w[None, :]).astype(np.float32)
    snk = (sin2 * k_norm_w[None, :]).astype(np.float32)
    return csq, snq, csk, snk


def _host_masks():
    idx = np.arange(P)
    stair = (idx[None, :] // BLOCK >= idx[:, None] // BLOCK).astype(NPBF16)
    return np.ascontiguousarray(stair)


def _host_x(xb):
    """[T, C] -> [NT, 128c, NC16, 128t] blocked-transposed bf16 layout."""
    return np.ascontiguousarray(
        xb.reshape(NT, P, NC16, P).transpose(0, 3, 2, 1).astype(NPBF16)
    )


_nc_cache = None


def kernel(x, W_attn, W_proj, q_norm_w, k_norm_w):
    global _nc_cache
    x = np.asarray(x, dtype=np.float32)
    W_attn = np.asarray(W_attn, dtype=np.float32)
    W_proj = np.asarray(W_proj, dtype=np.float32)
    q_norm_w = np.asarray(q_norm_w, dtype=np.float32)
    k_norm_w = np.asarray(k_norm_w, dtype=np.float32)
    B = x.shape[0]

    csq, snq, csk, snk = _host_tables(q_norm_w, k_norm_w)
    dm1 = _host_masks()
    xts = [_host_x(x[b]) for b in range(B)]

    in_maps = []
    for core in range(8):
        b, g = divmod(core, 4)
        wa_core = np.concatenate(
            [
                W_attn[:, g * QCOLS : (g + 1) * QCOLS],
                W_attn[:, C + g * HD : C + (g + 1) * HD],
                W_attn[:, C + N_KV * HD + g * HD : C + N_KV * HD + (g + 1) * HD],
            ],
            axis=1,
        )
        wp_core = W_proj[g * QCOLS : (g + 1) * QCOLS, :]
        in_maps.append(
            {
                "xt": xts[b],
                "wa": np.ascontiguousarray(wa_core.astype(NPBF16)),
                "wp": np.ascontiguousarray(wp_core.astype(NPBF16)),
                "csq": csq, "snq": snq, "csk": csk, "snk": snk,
                "dm1": dm1,
            }
        )

    if _nc_cache is None:
        _nc_cache = build_nc()
    res = run_bass_kernel_spmd(_nc_cache, in_maps, core_ids=list(range(8)))

    out = np.zeros((B, T, C), dtype=np.float32)
    for core in range(8):
        b = core // 4
        out[b] += res.results[core]["out"]
    return out



# revision 49
# speedup vs baseline: 3.7547x; 1.4119x over previous
"""Block-causal GQA attention layer on 8 Trainium2 NeuronCores.

Sharding: 8 cores = batch(2) x head-group(4). Core c handles batch b=c//4 and
head group g=c%4 (q heads 4g..4g+3, kv head g). W_attn is column-sharded by
head group, W_proj row-sharded; each core computes a partial [T, C] output and
the host sums the 4 partials per batch element.

Per-core device pipeline (all matmuls fp32r = full-rate tf32-like):
  B) software-pipelined over 16 t-chunks with a 1-chunk lag so the PE stream
     never waits on the ACT/DVE norm+rope chain:
       stage A(i): x DMA, PE-transpose x -> xT, QKV matmuls, RMS stats
       stage B(i-1): rs-scaled copyback (ACT), RoPE (DVE, negative-stride
       half-swap views, norm weights folded into host cos/sin tables),
       PE-transpose q,k -> qT,kT.
  C+D) per 512-wide T-block: scores sT = kT.T @ qT (block-causal lower tiles
     only), exp on ACT (scale=1/sqrt(d)), staircase mask on diagonal tiles,
     PV + all-ones denominator matmuls, approx-reciprocal normalize; then
     immediately the output projection for that T-block and its DMA out.
"""

import numpy as np

import concourse.bacc as bacc
import concourse.bass as bass
import concourse.tile as tile
import concourse.mybir as mybir
from concourse.bass_utils import run_bass_kernel_spmd
from concourse.masks import make_identity

P = 128
T = 2048
C = 2048
N_HEAD = 16
N_KV = 4
HD = 128          # head dim
HG = N_HEAD // N_KV  # heads per group = 4
BLOCK = 16
EPS = 1e-5
ROPE_BASE = 500000.0
QCOLS = HG * HD   # 512 q cols per core
JCOLS = QCOLS + 2 * HD  # 768 qkv cols per core
NT = T // P       # 16 t-chunks
NC16 = C // P     # 16 c-chunks
SCALE = 1.0 / float(np.sqrt(np.float32(HD)))

F32 = mybir.dt.float32
F32R = mybir.dt.float32r
BF16 = mybir.dt.bfloat16
NPBF16 = mybir.dt.np(mybir.dt.bfloat16)
AF = mybir.ActivationFunctionType
ALU = mybir.AluOpType


def build_nc():
    nc = bacc.Bacc("TRN2", target_bir_lowering=False)

    # xt[i, c, ci, t] = x[i*128 + t, ci*128 + c]: per-t-chunk pre-transposed
    # blocked layout prepared on host; one contiguous [128, 2048] DMA per chunk.
    xt = nc.dram_tensor("xt", [NT, P, NC16, P], BF16, kind="ExternalInput")
    wa = nc.dram_tensor("wa", [C, JCOLS], BF16, kind="ExternalInput")
    wp = nc.dram_tensor("wp", [QCOLS, C], BF16, kind="ExternalInput")
    # rope tables packed [csq | snq | csk | snk] along the free axis
    tbl = nc.dram_tensor("tbl", [T, 4 * HD], F32, kind="ExternalInput")
    dm1 = nc.dram_tensor("dm1", [P, P], BF16, kind="ExternalInput")
    out = nc.dram_tensor("out", [T, C], BF16, kind="ExternalOutput")

    with tile.TileContext(nc) as tc:
        with tc.tile_pool(name="persist", bufs=1) as persist:
            ident_f = persist.tile([P, P], F32)
            make_identity(nc, ident_f)
            ident = persist.tile([P, P], BF16)
            nc.vector.tensor_copy(ident, ident_f)
            rsk_all = persist.tile([P, NT], F32)
            ones = persist.tile([P, P], BF16)
            nc.vector.memset(ones, 1.0)
            dm1_sb = persist.tile([P, P], BF16)
            nc.sync.dma_start(dm1_sb, dm1[:])
            eps_sb = persist.tile([P, 1], F32)
            nc.vector.memset(eps_sb, EPS)

            qT = persist.tile([P, HG, T], BF16)     # [d, h, t]
            kT = persist.tile([P, T], BF16)         # [d, t]
            v_sb = persist.tile([P, NT, HD], BF16)  # [s_in_chunk, s_chunk, d']
            yT = persist.tile([P, HG, T], BF16)     # [d', h, t]
            wp_sb = persist.tile([P, HG, C], BF16)

            # ---------------- Phase B (pipelined, lag 1) -----------------
            with (
                tc.tile_pool(name="wts", bufs=1) as wts,
                tc.tile_pool(name="bstream", bufs=3) as bstream,
                tc.tile_pool(name="bwork", bufs=4) as bwork,
                tc.tile_pool(name="psB_tp", bufs=2, space="PSUM") as psB_tp,
                tc.tile_pool(name="psB_qa", bufs=3, space="PSUM") as psB_qa,
                tc.tile_pool(name="psB_qb", bufs=3, space="PSUM") as psB_qb,
            ):
                half = HD // 2
                st = {}       # chunk index -> stage-A state dict
                x_tiles = {}

                def dma_x(i):
                    xtl = bstream.tile([P, NC16, P], BF16, tag="x")
                    nc.sync.dma_start(xtl, xt[i])
                    x_tiles[i] = xtl

                wa_r = wa[:].rearrange("(co ci) j -> ci co j", ci=P)
                wa_tiles = [None] * NC16

                def load_wa(cis):
                    for ci in cis:
                        wa_ci = wts.tile([P, JCOLS], BF16, tag=f"wa{ci}", name=f"wa{ci}")
                        nc.sync.dma_start(wa_ci, wa_r[:, ci])
                        wa_tiles[ci] = wa_ci

                def stageB1(j):
                    """DVE rope on raw q/k straight from PSUM; ACT v copy.

                    The RMS scale rs is NOT applied here: rope commutes with a
                    per-row scalar, so rs_q (and 1/sqrt(d)) ride along in the
                    diagonal matrices used by the stageB2 transposes, and rs_k
                    is applied as the per-partition exp scale in phase C.
                    """
                    s = st[j]
                    qa_ps, qb_ps, rs = s["qa"], s["qb"], s["rs"]
                    nc.scalar.copy(v_sb[:, j, :], qb_ps[:, HD : 2 * HD])
                    nc.scalar.copy(rsk_all[:, j : j + 1], rs[:, HG : HG + 1])

                    t1q = bwork.tile([P, QCOLS], F32, tag="t1q")
                    t2q = bwork.tile([P, QCOLS], F32, tag="t2q")
                    snq_v = s["snq"].rearrange("p (s e) -> p s e", s=2)
                    for hh in range(HG):
                        h0 = hh * HD
                        # (q * rs_q) * cos  — rs folded into the rope mults
                        nc.vector.scalar_tensor_tensor(
                            t1q[:, h0 : h0 + HD],
                            qa_ps[:, h0 : h0 + HD],
                            rs[:, hh : hh + 1],
                            s["csq"],
                            ALU.mult,
                            ALU.mult,
                        )
                        qswp_h = bass.AP(
                            tensor=qa_ps.tensor,
                            offset=qa_ps.offset + h0 + half,
                            ap=[qa_ps.ap[0], [-half, 2], [1, half]],
                        )
                        nc.vector.scalar_tensor_tensor(
                            t2q[:, h0 : h0 + HD].rearrange(
                                "p (s e) -> p s e", s=2
                            ),
                            qswp_h,
                            rs[:, hh : hh + 1],
                            snq_v,
                            ALU.mult,
                            ALU.mult,
                        )
                    qhat = bwork.tile([P, QCOLS], BF16, tag="qhat")
                    nc.vector.tensor_tensor(qhat, t1q, t2q, ALU.add)

                    kswp = bass.AP(
                        tensor=qb_ps.tensor,
                        offset=qb_ps.offset + half,
                        ap=[qb_ps.ap[0], [-half, 2], [1, half]],
                    )
                    t1k = bwork.tile([P, HD], F32, tag="t1k")
                    nc.vector.tensor_tensor(
                        t1k, qb_ps[:, 0:HD], s["csk"], ALU.mult
                    )
                    t2k = bwork.tile([P, HD], F32, tag="t2k")
                    nc.vector.tensor_tensor(
                        t2k.rearrange("p (s e) -> p s e", s=2),
                        kswp,
                        s["snk"].rearrange("p (s e) -> p s e", s=2),
                        ALU.mult,
                    )
                    khat = bwork.tile([P, HD], BF16, tag="khat")
                    nc.vector.tensor_tensor(khat, t1k, t2k, ALU.add)
                    s["qhat"], s["khat"] = qhat, khat

                def stageB2(j):
                    """PE transposes of qhat/khat + copyback into qT/kT."""
                    s = st.pop(j)
                    t0 = j * P
                    tqk_ps = psB_tp.tile([P, 640], BF16, tag="tp")
                    for hh in range(HG):
                        nc.tensor.transpose(
                            tqk_ps[:, hh * HD : (hh + 1) * HD],
                            s["qhat"][:, hh * HD : (hh + 1) * HD],
                            ident,
                        )
                    nc.tensor.transpose(tqk_ps[:, QCOLS : QCOLS + HD], s["khat"], ident)
                    nc.vector.tensor_copy(
                        qT[:, :, t0 : t0 + P],
                        tqk_ps[:, 0:QCOLS].rearrange("p (h t) -> p h t", h=HG),
                    )
                    nc.vector.tensor_copy(kT[:, t0 : t0 + P], tqk_ps[:, QCOLS:640])

                def emit_tr(i):
                    # rope tables for chunk i (used in stage B1)
                    s = {}
                    t0 = i * P
                    tb = bstream.tile([P, 4 * HD], F32, tag="tbl", name="tbl_t")
                    nc.sync.dma_start(tb, tbl[t0 : t0 + P, :])
                    s["csq"] = tb[:, 0:HD]
                    s["snq"] = tb[:, HD : 2 * HD]
                    s["csk"] = tb[:, 2 * HD : 3 * HD]
                    s["snk"] = tb[:, 3 * HD : 4 * HD]
                    st[i] = s

                # startup DMA order: wa0 first so the chunk-0 QKV chain can
                # begin ASAP; x0/x1 and the first rope tables interleaved into
                # the wa stream so nothing downstream starves.
                load_wa([0])
                dma_x(0)
                emit_tr(0)
                load_wa([1, 2, 3])
                dma_x(1)
                emit_tr(1)
                load_wa([4, 5, 6, 7])
                dma_x(2)
                load_wa(range(8, NC16))
                for i in range(NT + 1):
                    if i < NT:
                        if i >= 2 and i + 1 < NT:
                            dma_x(i + 1)
                        if i == 12:
                            # prefetch wp so phase C isn't gated on it
                            nc.sync.dma_start(
                                wp_sb,
                                wp[:].rearrange("(h d) e -> d h e", d=P),
                            )

                        # premult + rope for chunk i-1 (ACT/DVE overlap the MMs)
                        if i >= 1:
                            stageB1(i - 1)

                        # QKV matmuls
                        s = st[i]
                        xT_sb = x_tiles.pop(i)
                        qa_ps = psB_qa.tile([P, QCOLS], F32, tag="qa")
                        qb_ps = psB_qb.tile([P, 2 * HD], F32, tag="qb")
                        for ci in range(NC16):
                            nc.tensor.matmul(
                                qa_ps,
                                xT_sb[:, ci],
                                wa_tiles[ci][:, 0:QCOLS],
                                start=(ci == 0),
                                stop=(ci == NC16 - 1),
                            )
                            nc.tensor.matmul(
                                qb_ps,
                                xT_sb[:, ci],
                                wa_tiles[ci][:, QCOLS:JCOLS],
                                start=(ci == 0),
                                stop=(ci == NC16 - 1),
                            )
                        s["qa"], s["qb"] = qa_ps, qb_ps

                        # rope tables for next chunk, then chunk i-1 q/k transposes
                        if i >= 1 and i + 1 < NT:
                            emit_tr(i + 1)
                        if i >= 1:
                            stageB2(i - 1)

                        # RMS stats
                        ss = bwork.tile([P, HG + 1], F32, tag="ss")
                        for hh in range(HG + 1):
                            src = (
                                qa_ps[:, hh * HD : (hh + 1) * HD]
                                if hh < HG
                                else qb_ps[:, 0:HD]
                            )
                            sq = bwork.tile([P, HD], F32, tag="sq")
                            nc.scalar.activation(
                                sq, src, AF.Square, accum_out=ss[:, hh : hh + 1]
                            )
                        rt = bwork.tile([P, HG + 1], F32, tag="rt")
                        nc.scalar.activation(
                            rt, ss, AF.Sqrt, bias=eps_sb, scale=1.0 / HD
                        )
                        rs = bwork.tile([P, HG + 1], F32, tag="rs")
                        nc.vector.reciprocal(rs, rt)
                        s["rs"] = rs
                    else:
                        stageB1(i - 1)
                        stageB2(i - 1)

            # ---------------- Phase C+D interleaved ----------------------
            # bf16 matmuls have no small-width penalty: all stages run on the
            # exact 128-granular staircase region.
            OFFS = [0, 128, 256, 384]
            if True:
                with (
                    tc.tile_pool(name="cwork", bufs=6) as cwork,
                    tc.tile_pool(name="dout", bufs=4) as dout,
                    tc.tile_pool(name="psC_acc", bufs=2, space="PSUM") as psC_acc,
                    tc.tile_pool(name="psC_sc", bufs=4, space="PSUM") as psC_sc,
                ):
                    def emit_proj_part(Tb, part):
                        tci = 4 * Tb + part
                        t0 = tci * P
                        o_sb = dout.tile([P, C], BF16, tag="o_sb")
                        for e in range(4):
                            o_ps = psC_sc.tile([P, 512], F32, tag="sc")
                            for h in range(HG):
                                nc.tensor.matmul(
                                    o_ps,
                                    yT[:, h, t0 : t0 + P],
                                    wp_sb[:, h, e * 512 : (e + 1) * 512],
                                    start=(h == 0),
                                    stop=(h == HG - 1),
                                )
                            nc.vector.tensor_copy(
                                o_sb[:, e * 512 : (e + 1) * 512], o_ps
                            )
                        nc.sync.dma_start(out[t0 : t0 + P, :], o_sb)

                    for Ti in range(4):
                        tt0 = Ti * 512
                        for h in range(HG):
                            if h == 1 and Ti >= 1:
                                for part in range(4):
                                    emit_proj_part(Ti - 1, part)
                            yt_ps = psC_acc.tile([P, 512], F32, tag="yt")
                            den_ps = psC_acc.tile([P, 512], F32, tag="den")
                            nS = 4 * Ti + 4
                            for S in range(nS):
                                r = S - 4 * Ti
                                off = OFFS[r] if r >= 0 else 0
                                sc_ps = psC_sc.tile([P, 512], F32, tag="sc")
                                nc.tensor.matmul(
                                    sc_ps[:, off:512],
                                    kT[:, S * P : (S + 1) * P],
                                    qT[:, h, tt0 + off : tt0 + 512],
                                    start=True,
                                    stop=True,
                                )
                                ex = cwork.tile([P, 512], BF16, tag="ex")
                                nc.scalar.activation(
                                    ex[:, off:512], sc_ps[:, off:512], AF.Exp,
                                    scale=rsk_all[:, S : S + 1],
                                )
                                if r >= 0:
                                    nc.vector.tensor_tensor(
                                        ex[:, off : off + P],
                                        ex[:, off : off + P],
                                        dm1_sb,
                                        ALU.mult,
                                    )
                                nc.tensor.matmul(
                                    yt_ps[:, off:512],
                                    v_sb[:, S, :],
                                    ex[:, off:512],
                                    start=(S == 0),
                                    stop=(S == nS - 1),
                                )
                                nc.tensor.matmul(
                                    den_ps[:, off:512],
                                    ones,
                                    ex[:, off:512],
                                    start=(S == 0),
                                    stop=(S == nS - 1),
                                )
                            denr = cwork.tile([P, 512], F32, tag="denr")
                            scr = cwork.tile([P, 512], F32, tag="scr")
                            nc.vector.reciprocal_approx_accurate(denr, den_ps, scr)
                            nc.vector.tensor_tensor(
                                yT[:, h, tt0 : tt0 + 512], yt_ps, denr, ALU.mult
                            )
                    for part in range(4):
                        emit_proj_part(3, part)



    nc.finalize()
    return nc


def _host_tables(q_norm_w, k_norm_w):
    """RoPE cos/sin tables in [t, d] layout with norm weights folded in."""
    half = HD // 2
    inv_freq = (
        1.0 / (ROPE_BASE ** (np.arange(0, half, dtype=np.float32) / half))
    ).astype(np.float32)
    ang = np.arange(T, dtype=np.float32)[:, None] * inv_freq[None, :]  # [T, half]
    cos = np.cos(ang).astype(np.float32)
    sin = np.sin(ang).astype(np.float32)
    cos2 = np.concatenate([cos, cos], axis=1)           # [T, 128]
    sin2 = np.concatenate([-sin, sin], axis=1)          # [T, 128]
    # q tables carry the softmax 1/sqrt(d) scale (rs_q rides in the rope
    # mults, rs_k in the exp scale)
    csq1 = cos2 * q_norm_w[None, :] * SCALE
    snq1 = sin2 * q_norm_w[None, :] * SCALE
    csq = np.ascontiguousarray(csq1, dtype=np.float32)  # [T, 128]
    snq = np.ascontiguousarray(snq1, dtype=np.float32)
    csk = (cos2 * k_norm_w[None, :]).astype(np.float32)
    snk = (sin2 * k_norm_w[None, :]).astype(np.float32)
    return np.ascontiguousarray(
        np.concatenate([csq, snq, csk, snk], axis=1)
    )


def _host_masks():
    idx = np.arange(P)
    stair = (idx[None, :] // BLOCK >= idx[:, None] // BLOCK).astype(NPBF16)
    return np.ascontiguousarray(stair)


def _host_x(xb):
    """[T, C] -> [NT, 128c, NC16, 128t] blocked-transposed bf16 layout."""
    return np.ascontiguousarray(
        xb.reshape(NT, P, NC16, P).transpose(0, 3, 2, 1).astype(NPBF16)
    )


_nc_cache = None


def kernel(x, W_attn, W_proj, q_norm_w, k_norm_w):
    global _nc_cache
    x = np.asarray(x, dtype=np.float32)
    W_attn = np.asarray(W_attn, dtype=np.float32)
    W_proj = np.asarray(W_proj, dtype=np.float32)
    q_norm_w = np.asarray(q_norm_w, dtype=np.float32)
    k_norm_w = np.asarray(k_norm_w, dtype=np.float32)
    B = x.shape[0]

    tblarr = _host_tables(q_norm_w, k_norm_w)
    dm1 = _host_masks()
    xts = [_host_x(x[b]) for b in range(B)]

    in_maps = []
    for core in range(8):
        b, g = divmod(core, 4)
        wa_core = np.concatenate(
            [
                W_attn[:, g * QCOLS : (g + 1) * QCOLS],
                W_attn[:, C + g * HD : C + (g + 1) * HD],
                W_attn[:, C + N_KV * HD + g * HD : C + N_KV * HD + (g + 1) * HD],
            ],
            axis=1,
        )
        wp_core = W_proj[g * QCOLS : (g + 1) * QCOLS, :]
        in_maps.append(
            {
                "xt": xts[b],
                "wa": np.ascontiguousarray(wa_core.astype(NPBF16)),
                "wp": np.ascontiguousarray(wp_core.astype(NPBF16)),
                "tbl": tblarr,
                "dm1": dm1,
            }
        )

    if _nc_cache is None:
        _nc_cache = build_nc()
    res = run_bass_kernel_spmd(_nc_cache, in_maps, core_ids=list(range(8)))

    out = np.zeros((B, T, C), dtype=np.float32)
    for core in range(8):
        b = core // 4
        out[b] += res.results[core]["out"].astype(np.float32)
    return out

